# revision 44
# baseline (speedup 1.0000x reference)
"""Trainium2 Bass kernel for nn_Attention_42279658062639 (sparse/topic attention).

Reference math (per batch b, head h):
    scores = q @ k^T / 8
    pair = mask_q * mask_k
    scores = where(pair, scores, -1e9)
    ts = tq @ tk^T / 8 ; ts = where(pair, ts, 1.0)
    tp = softmax(ts)                      (per batch, shared over heads)
    p_attn = softmax(scores * tp)
    out = p_attn @ v
    return (out, p_attn)

Device scheme (numpy- and CoreSim-validated):
  - 8 cores: core c -> (batch c//2, heads 8*(c%2) .. +8). No cross-core comm.
  - Mask compaction: only SC=640 gathered key columns (all ~515-530 valid ones
    plus masked padding) go on device; every elementwise pass, transpose, PV
    chunk and the p store shrink by ~37%. Host scatters p back to [q,1024]
    (masked columns are exactly 0 in the reference; fully-masked query rows are
    the constant 1/1024 row) -- pure data placement.
  - Host pre-transposes Q/K/topic to [d, s] layout with augment rows:
    qt row 64 = 1, kt row 64 = -1e9*(1-mask_k) so one matmul group yields
    s' = QK/8 + C.  QK and topic scores both run as split-bf16 hi/lo 3-term
    matmuls (~fp32 accuracy).
  - TP' = topic_probs * mask_q; x = s' * TP'; e = exp(x) on ACT with
    accumulated row-sum Z; p = e/Z.  Dead query rows: x=0 -> e=1; masked k in
    valid rows: x <= -2000 -> e=0 exactly like the reference.
  - p = e * (1/Z) runs on GPSIMD off the critical path (feeds only the p
    store).  The PV path transposes unnormalized e (PE, fp32, 128x128 tiles),
    evacuates PSUM->SBUF with an fp16 cast split between ACT and DVE, then
    out_un[q,d] = sum_c eT_c.T @ v_c on PE.  A 6th "fixup" PV chunk adds
    (1-mask_q) (x) sum(V over non-compact rows) so dead-row rows see the full
    sum(V); the [128,d] result is normalized by rz and a dead-row 640/1024
    rescale in one fused tensor_scalar.
  - Input loads ride the SP HWDGE ring, batched per-head stores the ACT ring.
"""

import sys
import numpy as np

try:
    import ml_dtypes
except ImportError:  # pragma: no cover
    sys.path.insert(0, "/opt/trn_rl_repo")
    import ml_dtypes

BF = ml_dtypes.bfloat16
B, H, S, D = 4, 16, 1024, 64
SC = 640               # compact key columns kept on device (multiple of 128)
NCORES = 8
HPC = H * B // NCORES  # heads per core = 8
NEG = -1.0e9
P = 128
E1 = float(np.exp(1.0))


def _concourse():
    try:
        import concourse.bass as bass  # noqa
    except ImportError:
        sys.path.insert(0, "/opt/trn_rl_repo")
    import concourse.bass as bass
    import concourse.tile as tile
    from concourse import mybir
    return bass, tile, mybir


def _bacc():
    _concourse()
    import concourse.bacc as bacc
    return bacc


def emit_kernel(tc, aps, hpc, s, d, sc):
    """Emit the whole per-core program. aps: dict name -> bass.AP."""
    bass, tile, mybir = _concourse()
    nc = tc.nc
    f32 = mybir.dt.float32
    bf16 = mybir.dt.bfloat16
    fp16 = mybir.dt.float16
    Alu = mybir.AluOpType
    Act = mybir.ActivationFunctionType

    nqt = s // P             # query tiles per head
    nkc = sc // P            # compact key chunks (transpose/PV granularity)
    nrest = (s - sc) // P    # leftover masked-key chunks (for the out fixup)
    mh = [(o, min(512, sc - o)) for o in range(0, sc, 512)]

    qt2_d, kt2_d, qt1_d, kt1_d, v_d = (
        aps["qt2"], aps["kt2"], aps["qt1"], aps["kt1"], aps["v"])
    vm_d = aps["vm"]
    tq2_d, tk2_d, tqh1_d, tkl_d = aps["tq2"], aps["tk2"], aps["tqh1"], aps["tkl"]
    mkrep_d, mqcol_d, ident_d = aps["mkrep"], aps["mqcol"], aps["ident"]
    deadrow_d, onescol_d, zsc_d = aps["deadrow"], aps["onescol"], aps["zsc"]
    p_out, o_out = aps["p_out"], aps["o_out"]

    from contextlib import ExitStack
    with ExitStack() as ctx:
        const_pool = ctx.enter_context(tc.tile_pool(name="const", bufs=1))
        tp_pool = ctx.enter_context(tc.tile_pool(name="tp", bufs=1))
        head_pool = ctx.enter_context(tc.tile_pool(name="head", bufs=2))
        x_pool = ctx.enter_context(tc.tile_pool(name="x", bufs=3))
        e_pool = ctx.enter_context(tc.tile_pool(name="e", bufs=3))
        p_pool = ctx.enter_context(tc.tile_pool(name="p", bufs=2))
        pt_pool = ctx.enter_context(tc.tile_pool(name="pt", bufs=3))
        o_pool = ctx.enter_context(tc.tile_pool(name="o", bufs=2))
        stat_pool = ctx.enter_context(tc.tile_pool(name="stat", bufs=6))
        # PSUM budget (8 banks): scores f32 [P,sc] 2 banks x2, pT-half f32
        # 1 bank x2, out f32 1 bank x2 (mv rides the o_ps tag slots).
        psum_s = ctx.enter_context(tc.tile_pool(name="psum_s", bufs=2, space="PSUM"))
        psum_t = ctx.enter_context(tc.tile_pool(name="psum_t", bufs=2, space="PSUM"))
        psum_o = ctx.enter_context(tc.tile_pool(name="psum_o", bufs=2, space="PSUM"))

        # ---- constants ----
        mkrep = const_pool.tile([P, sc], f32, tag="mkrep")
        nc.sync.dma_start(mkrep[:], mkrep_d[:])
        mqcol = const_pool.tile([P, nqt], f32, tag="mqcol")
        nc.sync.dma_start(mqcol[:], mqcol_d[:])
        zsc = const_pool.tile([P, nqt], f32, tag="zsc")
        nc.sync.dma_start(zsc[:], zsc_d[:])
        ident = const_pool.tile([P, P], f32, tag="ident")
        nc.sync.dma_start(ident[:], ident_d[:])
        deadrow = const_pool.tile([P, nqt, P], fp16, tag="deadrow")
        nc.sync.dma_start(deadrow[:], deadrow_d[:])
        onescol = const_pool.tile([P, 1], fp16, tag="onescol")
        nc.sync.dma_start(onescol[:], onescol_d[:])
        tq2 = const_pool.tile([P, s], bf16, tag="tq2")
        nc.sync.dma_start(tq2[:], tq2_d[:])
        tqh1 = const_pool.tile([d + 1, s], bf16, tag="tqh1")
        nc.sync.dma_start(tqh1[:], tqh1_d[:])
        tk2 = const_pool.tile([P, sc], bf16, tag="tk2")
        nc.sync.dma_start(tk2[:], tk2_d[:])
        tkl = const_pool.tile([d + 1, sc], bf16, tag="tkl")
        nc.sync.dma_start(tkl[:], tkl_d[:])

        # ---- topic probabilities (compact columns), shared by heads ----
        tp_tiles = []
        for qi in range(nqt):
            ts_ps = psum_s.tile([P, sc], f32, tag="ps")
            for (off, w) in mh:
                nc.tensor.matmul(ts_ps[:, off:off + w],
                                 lhsT=tq2[:, qi * P:(qi + 1) * P],
                                 rhs=tk2[:, off:off + w],
                                 start=True, stop=False)
                nc.tensor.matmul(ts_ps[:, off:off + w],
                                 lhsT=tqh1[:, qi * P:(qi + 1) * P],
                                 rhs=tkl[:, off:off + w],
                                 start=False, stop=True)
            w_t = x_pool.tile([P, sc], f32, tag="x")
            nc.vector.tensor_tensor(w_t[:], ts_ps[:], mkrep[:], Alu.mult)
            et = e_pool.tile([P, sc], f32, tag="e")
            zt = stat_pool.tile([P, 1], f32, tag="zt")
            nc.scalar.activation(et[:], w_t[:], Act.Exp,
                                 bias=1.0, scale=mqcol[:, qi:qi + 1],
                                 accum_out=zt[:])
            # reference Zt also sums e^1 over the (s-sc) non-compact columns
            zt2 = stat_pool.tile([P, 1], f32, tag="zt2")
            nc.vector.tensor_scalar_add(zt2[:], zt[:], float((s - sc) * E1))
            rzt = stat_pool.tile([P, 1], f32, tag="rzt")
            nc.vector.reciprocal(rzt[:], zt2[:])
            rztm = stat_pool.tile([P, 1], f32, tag="rztm")
            nc.vector.tensor_tensor(rztm[:], rzt[:], mqcol[:, qi:qi + 1], Alu.mult)
            tp_t = tp_pool.tile([P, sc], f32, tag=f"tp{qi}")
            nc.gpsimd.tensor_scalar_mul(tp_t[:], et[:], rztm[:])
            tp_tiles.append(tp_t)

        # ---- heads ----
        for hi in range(hpc):
            qt2_sb = head_pool.tile([P, s], bf16, tag="q2")
            nc.sync.dma_start(qt2_sb[:], qt2_d[hi])
            qt1_sb = head_pool.tile([d + 1, s], bf16, tag="q1")
            nc.sync.dma_start(qt1_sb[:], qt1_d[hi])
            kt2_sb = head_pool.tile([P, sc], bf16, tag="k2")
            nc.sync.dma_start(kt2_sb[:], kt2_d[hi])
            kt1_sb = head_pool.tile([d + 1, sc], bf16, tag="k1")
            nc.sync.dma_start(kt1_sb[:], kt1_d[hi])
            v_sb = head_pool.tile([P, nkc * d], fp16, tag="v")
            nc.sync.dma_start(v_sb[:], v_d[hi])

            # vmfix row 0 = sum of V over non-compact (masked, unpadded) rows
            vmfix = head_pool.tile([P, d], fp16, tag="vmfix")
            nc.gpsimd.memset(vmfix[:], 0.0)
            if nrest > 0:
                vm_sb = head_pool.tile([P, nrest * d], fp16, tag="vmrest")
                nc.sync.dma_start(vm_sb[:], vm_d[hi])
                mv_ps = psum_o.tile([1, d], f32, tag="o_ps")
                for c in range(nrest):
                    nc.tensor.matmul(mv_ps[:], lhsT=onescol[:],
                                     rhs=vm_sb[:, c * d:(c + 1) * d],
                                     start=(c == 0), stop=(c == nrest - 1))
                nc.scalar.copy(vmfix[0:1, :], mv_ps[:])

            p_head = p_pool.tile([P, nqt, sc], f32, tag="p")
            o_head = o_pool.tile([P, nqt, d], f32, tag="o")
            for qi in range(nqt):
                s_ps = psum_s.tile([P, sc], f32, tag="ps")
                for (off, w) in mh:
                    # qhi@khi + qlo@khi
                    nc.tensor.matmul(s_ps[:, off:off + w],
                                     lhsT=qt2_sb[:, qi * P:(qi + 1) * P],
                                     rhs=kt2_sb[:, off:off + w],
                                     start=True, stop=False)
                    # qhi@klo + C (aug row: ones x cK)
                    nc.tensor.matmul(s_ps[:, off:off + w],
                                     lhsT=qt1_sb[:, qi * P:(qi + 1) * P],
                                     rhs=kt1_sb[:, off:off + w],
                                     start=False, stop=True)
                x = x_pool.tile([P, sc], f32, tag="x")
                nc.vector.tensor_tensor(x[:], s_ps[:], tp_tiles[qi][:], Alu.mult)
                e = e_pool.tile([P, sc], f32, tag="e")
                z = stat_pool.tile([P, 1], f32, tag="z")
                nc.scalar.activation(e[:], x[:], Act.Exp,
                                     bias=0.0, scale=1.0, accum_out=z[:])
                rz = stat_pool.tile([P, 1], f32, tag="rz")
                nc.vector.reciprocal(rz[:], z[:])
                # p (for the p_attn output) is off the critical path:
                nc.gpsimd.tensor_scalar_mul(p_head[:, qi, :], e[:], rz[:])

                # PV path transposes unnormalized e; normalize on [128,d].
                ha = (nkc + 1) // 2
                pt = pt_pool.tile([P, nkc, P], fp16, tag="pt")
                for h2 in range(2):
                    c0, c1 = (0, ha) if h2 == 0 else (ha, nkc)
                    if c0 >= c1:
                        continue
                    pt_ps = psum_t.tile([P, ha, P], f32, tag="pt_ps")
                    for c in range(c0, c1):
                        nc.tensor.transpose(pt_ps[:, c - c0, :],
                                            e[:, c * P:(c + 1) * P], ident[:])
                    # PSUM f32 -> SBUF fp16 cast; balance ACT/DVE
                    if h2 == 0:
                        nc.scalar.copy(pt[:, c0:c1, :], pt_ps[:, 0:c1 - c0, :])
                    else:
                        nc.vector.tensor_copy(pt[:, c0:c1, :],
                                              pt_ps[:, 0:c1 - c0, :])
                o_ps = psum_o.tile([P, d], f32, tag="o_ps")
                for c in range(nkc):
                    nc.tensor.matmul(o_ps[:], lhsT=pt[:, c, :],
                                     rhs=v_sb[:, c * d:(c + 1) * d],
                                     start=(c == 0), stop=False)
                # dead-row fixup: (1-mask_q) outer sum(V over non-compact rows)
                nc.tensor.matmul(o_ps[:], lhsT=deadrow[:, qi, :], rhs=vmfix[:],
                                 start=False, stop=True)
                # o = o_ps * rz * zsc  (zsc rescales dead rows by sc/s)
                nc.vector.tensor_scalar(o_head[:, qi, :], o_ps[:],
                                        rz[:], zsc[:, qi:qi + 1],
                                        Alu.mult, Alu.mult)
            # one batched store per head for p_attn and out
            nc.scalar.dma_start(
                p_out[hi].rearrange("(t p) k -> p t k", p=P), p_head[:])
            nc.scalar.dma_start(
                o_out[hi].rearrange("(t p) k -> p t k", p=P), o_head[:])


def build_nc(hpc=HPC, s=S, d=D, sc=SC):
    bass, tile, mybir = _concourse()
    f32 = mybir.dt.float32
    bf16 = mybir.dt.bfloat16
    fp16 = mybir.dt.float16
    nqt = s // P
    nkc = sc // P
    nrest = (s - sc) // P

    bacc = _bacc()
    nc = bacc.Bacc(trn_type="TRN2", target_bir_lowering=False, debug=False)
    names = {
        "qt2": ([hpc, P, s], bf16, "ExternalInput"),
        "qt1": ([hpc, d + 1, s], bf16, "ExternalInput"),
        "kt2": ([hpc, P, sc], bf16, "ExternalInput"),
        "kt1": ([hpc, d + 1, sc], bf16, "ExternalInput"),
        "v": ([hpc, P, nkc * d], fp16, "ExternalInput"),
        "vm": ([hpc, P, max(nrest, 1) * d], fp16, "ExternalInput"),
        "tq2": ([P, s], bf16, "ExternalInput"),
        "tqh1": ([d + 1, s], bf16, "ExternalInput"),
        "tk2": ([P, sc], bf16, "ExternalInput"),
        "tkl": ([d + 1, sc], bf16, "ExternalInput"),
        "mkrep": ([P, sc], f32, "ExternalInput"),
        "mqcol": ([P, nqt], f32, "ExternalInput"),
        "zsc": ([P, nqt], f32, "ExternalInput"),
        "ident": ([P, P], f32, "ExternalInput"),
        "deadrow": ([P, nqt, P], fp16, "ExternalInput"),
        "onescol": ([P, 1], fp16, "ExternalInput"),
        "p_out": ([hpc, s, sc], f32, "ExternalOutput"),
        "o_out": ([hpc, s, d], f32, "ExternalOutput"),
    }
    aps = {}
    for name, (shape, dt_, kind) in names.items():
        aps[name] = nc.dram_tensor(name, shape, dt_, kind=kind).ap()

    with tile.TileContext(nc) as tc:
        emit_kernel(tc, aps, hpc, s, d, sc)
    nc.finalize()
    return nc


def compact_index(maskf, s, sc):
    """Gathered column order: valid keys, then masked padding to sc.
    Returns (idx[sc], rest[s-sc]) -- rest are masked keys not on device."""
    valid = np.nonzero(maskf > 0.5)[0]
    masked = np.nonzero(maskf <= 0.5)[0]
    npad = sc - len(valid)
    assert npad >= 0, f"valid count {len(valid)} exceeds SC={sc}"
    idx = np.concatenate([valid, masked[:npad]])
    rest = masked[npad:]
    return idx, rest


def make_core_inputs(q_c, k_c, v_c, maskf, tq_b, tk_b, s=S, d=D, sc=SC):
    """Host-side data prep for one core.
    q_c,k_c,v_c: [hpc, s, d] f32; maskf: [s] f32; tq_b,tk_b: [s, d] f32."""
    hpc = q_c.shape[0]
    nqt = s // P
    nkc = sc // P
    nrest = (s - sc) // P
    idx, rest = compact_index(maskf, s, sc)
    mko = maskf[idx]                                   # compact mask (k side)

    def hilo(a):
        hi = a.astype(BF)
        lo = (a - hi.astype(np.float32)).astype(BF)
        return hi, lo

    qT = (q_c / 8.0).transpose(0, 2, 1)                # [hpc, d, s]
    kTc = k_c[:, idx, :].transpose(0, 2, 1)            # [hpc, d, sc]
    qhi, qlo = hilo(qT)
    khi, klo = hilo(kTc)
    qt2 = np.ascontiguousarray(np.concatenate([qhi, qlo], 1))
    kt2 = np.ascontiguousarray(np.concatenate([khi, khi], 1))
    ones_h = np.ones((hpc, 1, s), BF)
    ck = np.broadcast_to((NEG * (1.0 - mko)).astype(BF), (hpc, 1, sc))
    qt1 = np.ascontiguousarray(np.concatenate([qhi, ones_h], 1))
    kt1 = np.ascontiguousarray(np.concatenate([klo, ck], 1))

    vv = v_c[:, idx, :].astype(np.float16).reshape(hpc, nkc, P, d)
    vv = np.ascontiguousarray(vv.transpose(0, 2, 1, 3)).reshape(hpc, P, nkc * d)
    if nrest > 0:
        vm = v_c[:, rest, :].astype(np.float16).reshape(hpc, nrest, P, d)
        vm = np.ascontiguousarray(vm.transpose(0, 2, 1, 3)).reshape(
            hpc, P, nrest * d)
    else:
        vm = np.zeros((hpc, P, d), np.float16)

    tqs = tq_b / 8.0
    tqhi, tqlo = hilo(tqs.T)                           # [d, s]
    tkc = tk_b[idx, :].T                               # [d, sc]
    tkhi, tklo = hilo(tkc)
    ones_r = np.ones((1, s), BF)
    tq2 = np.ascontiguousarray(np.concatenate([tqhi, tqlo], 0))
    tqh1 = np.ascontiguousarray(np.concatenate([tqhi, ones_r], 0))
    tk2 = np.ascontiguousarray(np.concatenate([tkhi, tkhi], 0))
    tkl = np.ascontiguousarray(
        np.concatenate([tklo, -np.ones((1, sc), BF)], 0))

    mkrep = np.ascontiguousarray(np.broadcast_to(mko, (P, sc))).astype(np.float32)
    mqcol = np.ascontiguousarray(maskf.reshape(nqt, P).T).astype(np.float32)
    zsc = (mqcol + (1.0 - mqcol) * (float(sc) / s)).astype(np.float32)
    ident = np.eye(P, dtype=np.float32)
    deadrow = np.zeros((P, nqt, P), np.float16)
    deadrow[0] = (1.0 - maskf.reshape(nqt, P)).astype(np.float16)
    onescol = np.ones((P, 1), np.float16)
    return {
        "qt2": qt2, "kt2": kt2, "qt1": qt1, "kt1": kt1, "v": vv, "vm": vm,
        "tq2": tq2, "tqh1": tqh1, "tk2": tk2, "tkl": tkl,
        "mkrep": mkrep, "mqcol": mqcol, "zsc": zsc, "ident": ident,
        "deadrow": deadrow, "onescol": onescol,
    }


def assemble_p(p_compact, maskf, s=S, sc=SC):
    """Scatter compact p back to full [.., s, s]; fill dead rows/columns.
    p_compact: [..., s, sc]."""
    idx, _ = compact_index(maskf, s, sc)
    full = np.zeros(p_compact.shape[:-1] + (s,), np.float32)
    full[..., idx] = p_compact
    dead = np.nonzero(maskf <= 0.5)[0]
    full[..., dead, :] = np.float32(1.0 / s)
    return full


_NC_CACHE = {}


def _get_nc():
    if "nc" not in _NC_CACHE:
        _NC_CACHE["nc"] = build_nc()
    return _NC_CACHE["nc"]


def run_cores(in_maps, trace=False, trace_kwargs=None):
    from concourse.bass_utils import run_bass_kernel_spmd
    nc = _get_nc()
    kw = {}
    if trace:
        kw["trace"] = True
        if trace_kwargs:
            kw["trace_kwargs"] = trace_kwargs
    return run_bass_kernel_spmd(nc, in_maps, list(range(NCORES)), **kw)


def kernel(query, key, value, head_nums, mask, topic_query, topic_key,
           _trace=False):
    query = np.asarray(query, np.float32)
    key = np.asarray(key, np.float32)
    value = np.asarray(value, np.float32)
    mask = np.asarray(mask)
    topic_query = np.asarray(topic_query, np.float32)
    topic_key = np.asarray(topic_key, np.float32)

    in_maps = []
    for c in range(NCORES):
        b = c // 2
        h0 = (c % 2) * HPC
        in_maps.append(make_core_inputs(
            query[b, h0:h0 + HPC], key[b, h0:h0 + HPC], value[b, h0:h0 + HPC],
            mask[b].astype(np.float32), topic_query[b], topic_key[b]))

    res = run_cores(in_maps, trace=_trace)
    out = np.zeros((B, H, S, D), np.float32)
    p_attn = np.zeros((B, H, S, S), np.float32)
    for c in range(NCORES):
        b = c // 2
        h0 = (c % 2) * HPC
        out[b, h0:h0 + HPC] = res.results[c]["o_out"]
        p_attn[b, h0:h0 + HPC] = assemble_p(
            res.results[c]["p_out"], mask[b].astype(np.float32))
    if _trace:
        return (out, p_attn), res
    return out, p_attn


# revision 48
# speedup vs baseline: 1.0067x; 1.0067x over previous
"""Trainium2 Bass kernel for nn_Attention_42279658062639 (sparse/topic attention).

Reference math (per batch b, head h):
    scores = q @ k^T / 8
    pair = mask_q * mask_k
    scores = where(pair, scores, -1e9)
    ts = tq @ tk^T / 8 ; ts = where(pair, ts, 1.0)
    tp = softmax(ts)                      (per batch, shared over heads)
    p_attn = softmax(scores * tp)
    out = p_attn @ v
    return (out, p_attn)

Device scheme (numpy- and CoreSim-validated):
  - 8 cores: core c -> (batch c//2, heads 8*(c%2) .. +8). No cross-core comm.
  - Mask compaction: only SC=640 gathered key columns (all ~515-530 valid ones
    plus masked padding) go on device; every elementwise pass, transpose, PV
    chunk and the p store shrink by ~37%. Host scatters p back to [q,1024]
    (masked columns are exactly 0 in the reference; fully-masked query rows are
    the constant 1/1024 row) -- pure data placement.
  - Host pre-transposes Q/K/topic to [d, s] layout with augment rows:
    qt row 64 = 1, kt row 64 = -1e9*(1-mask_k) so one matmul group yields
    s' = QK/8 + C.  QK and topic scores both run as split-bf16 hi/lo 3-term
    matmuls (~fp32 accuracy).
  - TP' = topic_probs * mask_q; x = s' * TP'; e = exp(x) on ACT with
    accumulated row-sum Z; p = e/Z.  Dead query rows: x=0 -> e=1; masked k in
    valid rows: x <= -2000 -> e=0 exactly like the reference.
  - p = e * (1/Z) runs on GPSIMD off the critical path (feeds only the p
    store).  The PV path transposes unnormalized e (PE, fp32, 128x128 tiles),
    evacuates PSUM->SBUF with an fp16 cast split between ACT and DVE, then
    out_un[q,d] = sum_c eT_c.T @ v_c on PE.  A 6th "fixup" PV chunk adds
    (1-mask_q) (x) sum(V over non-compact rows) so dead-row rows see the full
    sum(V); the [128,d] result is normalized by rz and a dead-row 640/1024
    rescale in one fused tensor_scalar.
  - Input loads ride the SP HWDGE ring, batched per-head stores the ACT ring.
"""

import sys
import numpy as np

try:
    import ml_dtypes
except ImportError:  # pragma: no cover
    sys.path.insert(0, "/opt/trn_rl_repo")
    import ml_dtypes

BF = ml_dtypes.bfloat16
B, H, S, D = 4, 16, 1024, 64
SC = 640               # compact key columns kept on device (multiple of 128)
NCORES = 8
HPC = H * B // NCORES  # heads per core = 8
NEG = -1.0e9
P = 128
E1 = float(np.exp(1.0))


def _concourse():
    try:
        import concourse.bass as bass  # noqa
    except ImportError:
        sys.path.insert(0, "/opt/trn_rl_repo")
    import concourse.bass as bass
    import concourse.tile as tile
    from concourse import mybir
    return bass, tile, mybir


def _bacc():
    _concourse()
    import concourse.bacc as bacc
    return bacc


def emit_kernel(tc, aps, hpc, s, d, sc):
    """Emit the whole per-core program. aps: dict name -> bass.AP."""
    bass, tile, mybir = _concourse()
    nc = tc.nc
    f32 = mybir.dt.float32
    bf16 = mybir.dt.bfloat16
    fp16 = mybir.dt.float16
    Alu = mybir.AluOpType
    Act = mybir.ActivationFunctionType

    nqt = s // P             # query tiles per head
    nkc = sc // P            # compact key chunks (transpose/PV granularity)
    nrest = (s - sc) // P    # leftover masked-key chunks (for the out fixup)
    mh = [(o, min(512, sc - o)) for o in range(0, sc, 512)]

    qt2_d, kt2_d, qt1_d, kt1_d, v_d = (
        aps["qt2"], aps["kt2"], aps["qt1"], aps["kt1"], aps["v"])
    vm_d = aps["vm"]
    tq2_d, tk2_d, tqh1_d, tkl_d = aps["tq2"], aps["tk2"], aps["tqh1"], aps["tkl"]
    mkrep_d, mqcol_d, ident_d = aps["mkrep"], aps["mqcol"], aps["ident"]
    deadrow_d, onescol_d, zsc_d = aps["deadrow"], aps["onescol"], aps["zsc"]
    p_out, o_out = aps["p_out"], aps["o_out"]

    from contextlib import ExitStack
    with ExitStack() as ctx:
        const_pool = ctx.enter_context(tc.tile_pool(name="const", bufs=1))
        tp_pool = ctx.enter_context(tc.tile_pool(name="tp", bufs=1))
        head_pool = ctx.enter_context(tc.tile_pool(name="head", bufs=2))
        x_pool = ctx.enter_context(tc.tile_pool(name="x", bufs=3))
        e_pool = ctx.enter_context(tc.tile_pool(name="e", bufs=3))
        p_pool = ctx.enter_context(tc.tile_pool(name="p", bufs=2))
        pt_pool = ctx.enter_context(tc.tile_pool(name="pt", bufs=3))
        o_pool = ctx.enter_context(tc.tile_pool(name="o", bufs=2))
        stat_pool = ctx.enter_context(tc.tile_pool(name="stat", bufs=6))
        # PSUM budget (8 banks): scores f32 [P,sc] 2 banks x2, pT-half f32
        # 1 bank x2, out f32 1 bank x2 (mv rides the o_ps tag slots).
        psum_sa = ctx.enter_context(tc.tile_pool(name="psum_sa", bufs=3, space="PSUM"))
        psum_sb = ctx.enter_context(tc.tile_pool(name="psum_sb", bufs=2, space="PSUM"))
        psum_t = ctx.enter_context(tc.tile_pool(name="psum_t", bufs=2, space="PSUM"))
        psum_o = ctx.enter_context(tc.tile_pool(name="psum_o", bufs=1, space="PSUM"))

        # ---- constants ----
        mkrep = const_pool.tile([P, sc], f32, tag="mkrep")
        nc.sync.dma_start(mkrep[:], mkrep_d[:])
        mqcol = const_pool.tile([P, nqt], f32, tag="mqcol")
        nc.sync.dma_start(mqcol[:], mqcol_d[:])
        zsc = const_pool.tile([P, nqt], f32, tag="zsc")
        nc.sync.dma_start(zsc[:], zsc_d[:])
        ident = const_pool.tile([P, P], f32, tag="ident")
        nc.sync.dma_start(ident[:], ident_d[:])
        deadrow = const_pool.tile([P, nqt, P], fp16, tag="deadrow")
        nc.sync.dma_start(deadrow[:], deadrow_d[:])
        onescol = const_pool.tile([P, 1], fp16, tag="onescol")
        nc.sync.dma_start(onescol[:], onescol_d[:])
        tq2 = const_pool.tile([P, s], bf16, tag="tq2")
        nc.sync.dma_start(tq2[:], tq2_d[:])
        tqh1 = const_pool.tile([d + 1, s], bf16, tag="tqh1")
        nc.sync.dma_start(tqh1[:], tqh1_d[:])
        tk2 = const_pool.tile([P, sc], bf16, tag="tk2")
        nc.sync.dma_start(tk2[:], tk2_d[:])
        tkl = const_pool.tile([d + 1, sc], bf16, tag="tkl")
        nc.sync.dma_start(tkl[:], tkl_d[:])

        # ---- topic probabilities (compact columns), shared by heads ----
        tp_tiles = []
        for qi in range(nqt):
            halves = []
            w_t = x_pool.tile([P, sc], f32, tag="x")
            for (off, w) in mh:
                hp = (psum_sa if off == 0 else psum_sb).tile(
                    [P, w], f32, tag="psa" if off == 0 else "psb")
                nc.tensor.matmul(hp[:],
                                 lhsT=tq2[:, qi * P:(qi + 1) * P],
                                 rhs=tk2[:, off:off + w],
                                 start=True, stop=False)
                nc.tensor.matmul(hp[:],
                                 lhsT=tqh1[:, qi * P:(qi + 1) * P],
                                 rhs=tkl[:, off:off + w],
                                 start=False, stop=True)
                nc.vector.tensor_tensor(w_t[:, off:off + w], hp[:],
                                        mkrep[:, off:off + w], Alu.mult)
            et = e_pool.tile([P, sc], f32, tag="e")
            zt = stat_pool.tile([P, 1], f32, tag="zt")
            nc.scalar.activation(et[:], w_t[:], Act.Exp,
                                 bias=1.0, scale=mqcol[:, qi:qi + 1],
                                 accum_out=zt[:])
            # reference Zt also sums e^1 over the (s-sc) non-compact columns
            zt2 = stat_pool.tile([P, 1], f32, tag="zt2")
            nc.vector.tensor_scalar_add(zt2[:], zt[:], float((s - sc) * E1))
            rzt = stat_pool.tile([P, 1], f32, tag="rzt")
            nc.vector.reciprocal(rzt[:], zt2[:])
            rztm = stat_pool.tile([P, 1], f32, tag="rztm")
            nc.vector.tensor_tensor(rztm[:], rzt[:], mqcol[:, qi:qi + 1], Alu.mult)
            tp_t = tp_pool.tile([P, sc], f32, tag=f"tp{qi}")
            nc.gpsimd.tensor_scalar_mul(tp_t[:], et[:], rztm[:])
            tp_tiles.append(tp_t)

        # ---- heads ----
        for hi in range(hpc):
            qt2_sb = head_pool.tile([P, s], bf16, tag="q2")
            nc.sync.dma_start(qt2_sb[:], qt2_d[hi])
            qt1_sb = head_pool.tile([d + 1, s], bf16, tag="q1")
            nc.sync.dma_start(qt1_sb[:], qt1_d[hi])
            kt2_sb = head_pool.tile([P, sc], bf16, tag="k2")
            nc.sync.dma_start(kt2_sb[:], kt2_d[hi])
            kt1_sb = head_pool.tile([d + 1, sc], bf16, tag="k1")
            nc.sync.dma_start(kt1_sb[:], kt1_d[hi])
            v_sb = head_pool.tile([P, nkc * d], fp16, tag="v")
            nc.sync.dma_start(v_sb[:], v_d[hi])

            # vmfix row 0 = sum of V over non-compact (masked, unpadded) rows
            vmfix = head_pool.tile([P, d], fp16, tag="vmfix")
            nc.gpsimd.memset(vmfix[:], 0.0)
            if nrest > 0:
                vm_sb = head_pool.tile([P, nrest * d], fp16, tag="vmrest")
                nc.sync.dma_start(vm_sb[:], vm_d[hi])
                mv_ps = psum_o.tile([1, d], f32, tag="o_ps")
                for c in range(nrest):
                    nc.tensor.matmul(mv_ps[:], lhsT=onescol[:],
                                     rhs=vm_sb[:, c * d:(c + 1) * d],
                                     start=(c == 0), stop=(c == nrest - 1))
                nc.scalar.copy(vmfix[0:1, :], mv_ps[:])

            p_head = p_pool.tile([P, nqt, sc], f32, tag="p")
            o_head = o_pool.tile([P, nqt, d], f32, tag="o")

            def tile_front(qi):
                halves = []
                for (off, w) in mh:
                    hp = (psum_sa if off == 0 else psum_sb).tile(
                        [P, w], f32, tag="psa" if off == 0 else "psb")
                    # qhi@khi + qlo@khi
                    nc.tensor.matmul(hp[:],
                                     lhsT=qt2_sb[:, qi * P:(qi + 1) * P],
                                     rhs=kt2_sb[:, off:off + w],
                                     start=True, stop=False)
                    # qhi@klo + C (aug row: ones x cK)
                    nc.tensor.matmul(hp[:],
                                     lhsT=qt1_sb[:, qi * P:(qi + 1) * P],
                                     rhs=kt1_sb[:, off:off + w],
                                     start=False, stop=True)
                    halves.append((off, w, hp))
                return halves

            def tile_back(qi, halves):
                x = x_pool.tile([P, sc], f32, tag="x")
                for (off, w, hp) in halves:
                    nc.vector.tensor_tensor(x[:, off:off + w], hp[:],
                                            tp_tiles[qi][:, off:off + w],
                                            Alu.mult)
                e = e_pool.tile([P, sc], f32, tag="e")
                z = stat_pool.tile([P, 1], f32, tag="z")
                nc.scalar.activation(e[:], x[:], Act.Exp,
                                     bias=0.0, scale=1.0, accum_out=z[:])
                rz = stat_pool.tile([P, 1], f32, tag="rz")
                nc.vector.reciprocal(rz[:], z[:])
                # p (for the p_attn output) is off the critical path:
                nc.gpsimd.tensor_scalar_mul(p_head[:, qi, :], e[:], rz[:])

                # PV path transposes unnormalized e; normalize on [128,d].
                ha = (nkc + 1) // 2
                pt = pt_pool.tile([P, nkc, P], fp16, tag="pt")
                for h2 in range(2):
                    c0, c1 = (0, ha) if h2 == 0 else (ha, nkc)
                    if c0 >= c1:
                        continue
                    pt_ps = psum_t.tile([P, ha, P], f32, tag="pt_ps")
                    for c in range(c0, c1):
                        nc.tensor.transpose(pt_ps[:, c - c0, :],
                                            e[:, c * P:(c + 1) * P], ident[:])
                    # PSUM f32 -> SBUF fp16 cast; balance ACT/DVE
                    if h2 == 0:
                        nc.scalar.copy(pt[:, c0:c1, :], pt_ps[:, 0:c1 - c0, :])
                    else:
                        nc.vector.tensor_copy(pt[:, c0:c1, :],
                                              pt_ps[:, 0:c1 - c0, :])
                o_ps = psum_o.tile([P, d], f32, tag="o_ps")
                for c in range(nkc):
                    nc.tensor.matmul(o_ps[:], lhsT=pt[:, c, :],
                                     rhs=v_sb[:, c * d:(c + 1) * d],
                                     start=(c == 0), stop=False)
                # dead-row fixup: (1-mask_q) outer sum(V over non-compact rows)
                nc.tensor.matmul(o_ps[:], lhsT=deadrow[:, qi, :], rhs=vmfix[:],
                                 start=False, stop=True)
                # o = o_ps * rz * zsc  (zsc rescales dead rows by sc/s)
                nc.vector.tensor_scalar(o_head[:, qi, :], o_ps[:],
                                        rz[:], zsc[:, qi:qi + 1],
                                        Alu.mult, Alu.mult)

            # 1-deep software pipeline: QK(qi+1) issues before back(qi) so
            # the in-order PE stream never waits on ACT's exp.
            prev = None
            for qi in range(nqt):
                sp = tile_front(qi)
                if prev is not None:
                    tile_back(*prev)
                prev = (qi, sp)
            tile_back(*prev)
            # one batched store per head for p_attn and out
            nc.gpsimd.dma_start(
                p_out[hi].rearrange("(t p) k -> p t k", p=P), p_head[:])
            nc.scalar.dma_start(
                o_out[hi].rearrange("(t p) k -> p t k", p=P), o_head[:])


def build_nc(hpc=HPC, s=S, d=D, sc=SC):
    bass, tile, mybir = _concourse()
    f32 = mybir.dt.float32
    bf16 = mybir.dt.bfloat16
    fp16 = mybir.dt.float16
    nqt = s // P
    nkc = sc // P
    nrest = (s - sc) // P

    bacc = _bacc()
    nc = bacc.Bacc(trn_type="TRN2", target_bir_lowering=False, debug=False)
    names = {
        "qt2": ([hpc, P, s], bf16, "ExternalInput"),
        "qt1": ([hpc, d + 1, s], bf16, "ExternalInput"),
        "kt2": ([hpc, P, sc], bf16, "ExternalInput"),
        "kt1": ([hpc, d + 1, sc], bf16, "ExternalInput"),
        "v": ([hpc, P, nkc * d], fp16, "ExternalInput"),
        "vm": ([hpc, P, max(nrest, 1) * d], fp16, "ExternalInput"),
        "tq2": ([P, s], bf16, "ExternalInput"),
        "tqh1": ([d + 1, s], bf16, "ExternalInput"),
        "tk2": ([P, sc], bf16, "ExternalInput"),
        "tkl": ([d + 1, sc], bf16, "ExternalInput"),
        "mkrep": ([P, sc], f32, "ExternalInput"),
        "mqcol": ([P, nqt], f32, "ExternalInput"),
        "zsc": ([P, nqt], f32, "ExternalInput"),
        "ident": ([P, P], f32, "ExternalInput"),
        "deadrow": ([P, nqt, P], fp16, "ExternalInput"),
        "onescol": ([P, 1], fp16, "ExternalInput"),
        "p_out": ([hpc, s, sc], f32, "ExternalOutput"),
        "o_out": ([hpc, s, d], f32, "ExternalOutput"),
    }
    aps = {}
    for name, (shape, dt_, kind) in names.items():
        aps[name] = nc.dram_tensor(name, shape, dt_, kind=kind).ap()

    with tile.TileContext(nc) as tc:
        emit_kernel(tc, aps, hpc, s, d, sc)
    nc.finalize()
    return nc


def compact_index(maskf, s, sc):
    """Gathered column order: valid keys, then masked padding to sc.
    Returns (idx[sc], rest[s-sc]) -- rest are masked keys not on device."""
    valid = np.nonzero(maskf > 0.5)[0]
    masked = np.nonzero(maskf <= 0.5)[0]
    npad = sc - len(valid)
    assert npad >= 0, f"valid count {len(valid)} exceeds SC={sc}"
    idx = np.concatenate([valid, masked[:npad]])
    rest = masked[npad:]
    return idx, rest


def make_core_inputs(q_c, k_c, v_c, maskf, tq_b, tk_b, s=S, d=D, sc=SC):
    """Host-side data prep for one core.
    q_c,k_c,v_c: [hpc, s, d] f32; maskf: [s] f32; tq_b,tk_b: [s, d] f32."""
    hpc = q_c.shape[0]
    nqt = s // P
    nkc = sc // P
    nrest = (s - sc) // P
    idx, rest = compact_index(maskf, s, sc)
    mko = maskf[idx]                                   # compact mask (k side)

    def hilo(a):
        hi = a.astype(BF)
        lo = (a - hi.astype(np.float32)).astype(BF)
        return hi, lo

    qT = (q_c / 8.0).transpose(0, 2, 1)                # [hpc, d, s]
    kTc = k_c[:, idx, :].transpose(0, 2, 1)            # [hpc, d, sc]
    qhi, qlo = hilo(qT)
    khi, klo = hilo(kTc)
    qt2 = np.ascontiguousarray(np.concatenate([qhi, qlo], 1))
    kt2 = np.ascontiguousarray(np.concatenate([khi, khi], 1))
    ones_h = np.ones((hpc, 1, s), BF)
    ck = np.broadcast_to((NEG * (1.0 - mko)).astype(BF), (hpc, 1, sc))
    qt1 = np.ascontiguousarray(np.concatenate([qhi, ones_h], 1))
    kt1 = np.ascontiguousarray(np.concatenate([klo, ck], 1))

    vv = v_c[:, idx, :].astype(np.float16).reshape(hpc, nkc, P, d)
    vv = np.ascontiguousarray(vv.transpose(0, 2, 1, 3)).reshape(hpc, P, nkc * d)
    if nrest > 0:
        vm = v_c[:, rest, :].astype(np.float16).reshape(hpc, nrest, P, d)
        vm = np.ascontiguousarray(vm.transpose(0, 2, 1, 3)).reshape(
            hpc, P, nrest * d)
    else:
        vm = np.zeros((hpc, P, d), np.float16)

    tqs = tq_b / 8.0
    tqhi, tqlo = hilo(tqs.T)                           # [d, s]
    tkc = tk_b[idx, :].T                               # [d, sc]
    tkhi, tklo = hilo(tkc)
    ones_r = np.ones((1, s), BF)
    tq2 = np.ascontiguousarray(np.concatenate([tqhi, tqlo], 0))
    tqh1 = np.ascontiguousarray(np.concatenate([tqhi, ones_r], 0))
    tk2 = np.ascontiguousarray(np.concatenate([tkhi, tkhi], 0))
    tkl = np.ascontiguousarray(
        np.concatenate([tklo, -np.ones((1, sc), BF)], 0))

    mkrep = np.ascontiguousarray(np.broadcast_to(mko, (P, sc))).astype(np.float32)
    mqcol = np.ascontiguousarray(maskf.reshape(nqt, P).T).astype(np.float32)
    zsc = (mqcol + (1.0 - mqcol) * (float(sc) / s)).astype(np.float32)
    ident = np.eye(P, dtype=np.float32)
    deadrow = np.zeros((P, nqt, P), np.float16)
    deadrow[0] = (1.0 - maskf.reshape(nqt, P)).astype(np.float16)
    onescol = np.ones((P, 1), np.float16)
    return {
        "qt2": qt2, "kt2": kt2, "qt1": qt1, "kt1": kt1, "v": vv, "vm": vm,
        "tq2": tq2, "tqh1": tqh1, "tk2": tk2, "tkl": tkl,
        "mkrep": mkrep, "mqcol": mqcol, "zsc": zsc, "ident": ident,
        "deadrow": deadrow, "onescol": onescol,
    }


def assemble_p(p_compact, maskf, s=S, sc=SC):
    """Scatter compact p back to full [.., s, s]; fill dead rows/columns.
    p_compact: [..., s, sc]."""
    idx, _ = compact_index(maskf, s, sc)
    full = np.zeros(p_compact.shape[:-1] + (s,), np.float32)
    full[..., idx] = p_compact
    dead = np.nonzero(maskf <= 0.5)[0]
    full[..., dead, :] = np.float32(1.0 / s)
    return full


_NC_CACHE = {}


def _get_nc():
    if "nc" not in _NC_CACHE:
        _NC_CACHE["nc"] = build_nc()
    return _NC_CACHE["nc"]


def run_cores(in_maps, trace=False, trace_kwargs=None):
    from concourse.bass_utils import run_bass_kernel_spmd
    nc = _get_nc()
    kw = {}
    if trace:
        kw["trace"] = True
        if trace_kwargs:
            kw["trace_kwargs"] = trace_kwargs
    return run_bass_kernel_spmd(nc, in_maps, list(range(NCORES)), **kw)


def kernel(query, key, value, head_nums, mask, topic_query, topic_key,
           _trace=False):
    query = np.asarray(query, np.float32)
    key = np.asarray(key, np.float32)
    value = np.asarray(value, np.float32)
    mask = np.asarray(mask)
    topic_query = np.asarray(topic_query, np.float32)
    topic_key = np.asarray(topic_key, np.float32)

    in_maps = []
    for c in range(NCORES):
        b = c // 2
        h0 = (c % 2) * HPC
        in_maps.append(make_core_inputs(
            query[b, h0:h0 + HPC], key[b, h0:h0 + HPC], value[b, h0:h0 + HPC],
            mask[b].astype(np.float32), topic_query[b], topic_key[b]))

    res = run_cores(in_maps, trace=_trace)
    out = np.zeros((B, H, S, D), np.float32)
    p_attn = np.zeros((B, H, S, S), np.float32)
    for c in range(NCORES):
        b = c // 2
        h0 = (c % 2) * HPC
        out[b, h0:h0 + HPC] = res.results[c]["o_out"]
        p_attn[b, h0:h0 + HPC] = assemble_p(
            res.results[c]["p_out"], mask[b].astype(np.float32))
    if _trace:
        return (out, p_attn), res
    return out, p_attn


# revision 52
# speedup vs baseline: 1.0339x; 1.0270x over previous
"""Trainium2 Bass kernel for nn_Attention_42279658062639 (sparse/topic attention).

Reference math (per batch b, head h):
    scores = q @ k^T / 8
    pair = mask_q * mask_k
    scores = where(pair, scores, -1e9)
    ts = tq @ tk^T / 8 ; ts = where(pair, ts, 1.0)
    tp = softmax(ts)                      (per batch, shared over heads)
    p_attn = softmax(scores * tp)
    out = p_attn @ v
    return (out, p_attn)

Device scheme (numpy- and CoreSim-validated):
  - 8 cores: core c -> (batch c//2, heads 8*(c%2) .. +8). No cross-core comm.
  - Mask compaction: only SC=640 gathered key columns (all ~515-530 valid ones
    plus masked padding) go on device; every elementwise pass, transpose, PV
    chunk and the p store shrink by ~37%. Host scatters p back to [q,1024]
    (masked columns are exactly 0 in the reference; fully-masked query rows are
    the constant 1/1024 row) -- pure data placement.
  - Host pre-transposes Q/K/topic to [d, s] layout with augment rows:
    qt row 64 = 1, kt row 64 = -1e9*(1-mask_k) so one matmul group yields
    s' = QK/8 + C.  QK and topic scores both run as split-bf16 hi/lo 3-term
    matmuls (~fp32 accuracy).
  - TP' = topic_probs * mask_q; x = s' * TP'; e = exp(x) on ACT with
    accumulated row-sum Z; p = e/Z.  Dead query rows: x=0 -> e=1; masked k in
    valid rows: x <= -2000 -> e=0 exactly like the reference.
  - p = e * (1/Z) runs on GPSIMD off the critical path (feeds only the p
    store).  The PV path transposes unnormalized e (PE, fp32, 128x128 tiles),
    evacuates PSUM->SBUF with an fp16 cast split between ACT and DVE, then
    out_un[q,d] = sum_c eT_c.T @ v_c on PE.  A 6th "fixup" PV chunk adds
    (1-mask_q) (x) sum(V over non-compact rows) so dead-row rows see the full
    sum(V); the [128,d] result is normalized by rz and a dead-row 640/1024
    rescale in one fused tensor_scalar.
  - Input loads ride the SP HWDGE ring, batched per-head stores the ACT ring.
"""

import sys
import numpy as np

try:
    import ml_dtypes
except ImportError:  # pragma: no cover
    sys.path.insert(0, "/opt/trn_rl_repo")
    import ml_dtypes

BF = ml_dtypes.bfloat16
B, H, S, D = 4, 16, 1024, 64
SC = 640               # compact key columns kept on device (multiple of 128)
NCORES = 8
HPC = H * B // NCORES  # heads per core = 8
NEG = -1.0e9
P = 128
E1 = float(np.exp(1.0))


def _concourse():
    try:
        import concourse.bass as bass  # noqa
    except ImportError:
        sys.path.insert(0, "/opt/trn_rl_repo")
    import concourse.bass as bass
    import concourse.tile as tile
    from concourse import mybir
    return bass, tile, mybir


def _bacc():
    _concourse()
    import concourse.bacc as bacc
    return bacc


def emit_kernel(tc, aps, hpc, s, d, sc):
    """Emit the whole per-core program. aps: dict name -> bass.AP."""
    bass, tile, mybir = _concourse()
    nc = tc.nc
    f32 = mybir.dt.float32
    bf16 = mybir.dt.bfloat16
    fp16 = mybir.dt.float16
    Alu = mybir.AluOpType
    Act = mybir.ActivationFunctionType

    nqt = s // P             # query tiles per head
    nkc = sc // P            # compact key chunks (transpose/PV granularity)
    nrest = (s - sc) // P    # leftover masked-key chunks (for the out fixup)
    mh = [(o, min(512, sc - o)) for o in range(0, sc, 512)]

    qt2_d, kt2_d, qt1_d, kt1_d, v_d = (
        aps["qt2"], aps["kt2"], aps["qt1"], aps["kt1"], aps["v"])
    vm_d = aps["vm"]
    tq2_d, tk2_d, tqh1_d, tkl_d = aps["tq2"], aps["tk2"], aps["tqh1"], aps["tkl"]
    mkrep_d, mqcol_d, ident_d = aps["mkrep"], aps["mqcol"], aps["ident"]
    deadrow_d, onescol_d, zsc_d = aps["deadrow"], aps["onescol"], aps["zsc"]
    p_out, o_out = aps["p_out"], aps["o_out"]

    from contextlib import ExitStack
    with ExitStack() as ctx:
        const_pool = ctx.enter_context(tc.tile_pool(name="const", bufs=1))
        tp_pool = ctx.enter_context(tc.tile_pool(name="tp", bufs=1))
        head_pool = ctx.enter_context(tc.tile_pool(name="head", bufs=2))
        x_pool = ctx.enter_context(tc.tile_pool(name="x", bufs=3))
        e_pool = ctx.enter_context(tc.tile_pool(name="e", bufs=3))
        p_pool = ctx.enter_context(tc.tile_pool(name="p", bufs=2))
        pt_pool = ctx.enter_context(tc.tile_pool(name="pt", bufs=3))
        o_pool = ctx.enter_context(tc.tile_pool(name="o", bufs=2))
        stat_pool = ctx.enter_context(tc.tile_pool(name="stat", bufs=6))
        # PSUM budget (8 banks): scores f32 [P,sc] 2 banks x2, pT-half f32
        # 1 bank x2, out f32 1 bank x2 (mv rides the o_ps tag slots).
        psum_s = ctx.enter_context(tc.tile_pool(name="psum_s", bufs=2, space="PSUM"))
        psum_t = ctx.enter_context(tc.tile_pool(name="psum_t", bufs=2, space="PSUM"))
        psum_o = ctx.enter_context(tc.tile_pool(name="psum_o", bufs=2, space="PSUM"))

        # ---- constants ----
        mkrep = const_pool.tile([P, sc], f32, tag="mkrep")
        nc.sync.dma_start(mkrep[:], mkrep_d[:])
        mqcol = const_pool.tile([P, nqt], f32, tag="mqcol")
        nc.sync.dma_start(mqcol[:], mqcol_d[:])
        zsc = const_pool.tile([P, nqt], f32, tag="zsc")
        nc.sync.dma_start(zsc[:], zsc_d[:])
        ident = const_pool.tile([P, P], f32, tag="ident")
        nc.sync.dma_start(ident[:], ident_d[:])
        deadrow = const_pool.tile([P, nqt, P], fp16, tag="deadrow")
        nc.sync.dma_start(deadrow[:], deadrow_d[:])
        onescol = const_pool.tile([P, 1], fp16, tag="onescol")
        nc.sync.dma_start(onescol[:], onescol_d[:])
        tq2 = const_pool.tile([P, s], bf16, tag="tq2")
        nc.sync.dma_start(tq2[:], tq2_d[:])
        tqh1 = const_pool.tile([d + 1, s], bf16, tag="tqh1")
        nc.sync.dma_start(tqh1[:], tqh1_d[:])
        tk2 = const_pool.tile([P, sc], bf16, tag="tk2")
        nc.sync.dma_start(tk2[:], tk2_d[:])
        tkl = const_pool.tile([d + 1, sc], bf16, tag="tkl")
        nc.sync.dma_start(tkl[:], tkl_d[:])

        # ---- topic probabilities (compact columns), shared by heads ----
        tp_tiles = []
        for qi in range(nqt):
            ts_ps = psum_s.tile([P, sc], f32, tag="ps")
            for (off, w) in mh:
                nc.tensor.matmul(ts_ps[:, off:off + w],
                                 lhsT=tq2[:, qi * P:(qi + 1) * P],
                                 rhs=tk2[:, off:off + w],
                                 start=True, stop=False)
                nc.tensor.matmul(ts_ps[:, off:off + w],
                                 lhsT=tqh1[:, qi * P:(qi + 1) * P],
                                 rhs=tkl[:, off:off + w],
                                 start=False, stop=True)
            w_t = x_pool.tile([P, sc], f32, tag="x")
            nc.vector.tensor_tensor(w_t[:], ts_ps[:], mkrep[:], Alu.mult)
            et = e_pool.tile([P, sc], f32, tag="e")
            zt = stat_pool.tile([P, 1], f32, tag="zt")
            nc.scalar.activation(et[:], w_t[:], Act.Exp,
                                 bias=1.0, scale=mqcol[:, qi:qi + 1],
                                 accum_out=zt[:])
            # reference Zt also sums e^1 over the (s-sc) non-compact columns
            zt2 = stat_pool.tile([P, 1], f32, tag="zt2")
            nc.vector.tensor_scalar_add(zt2[:], zt[:], float((s - sc) * E1))
            rzt = stat_pool.tile([P, 1], f32, tag="rzt")
            nc.vector.reciprocal(rzt[:], zt2[:])
            rztm = stat_pool.tile([P, 1], f32, tag="rztm")
            nc.vector.tensor_tensor(rztm[:], rzt[:], mqcol[:, qi:qi + 1], Alu.mult)
            tp_t = tp_pool.tile([P, sc], f32, tag=f"tp{qi}")
            nc.gpsimd.tensor_scalar_mul(tp_t[:], et[:], rztm[:])
            tp_tiles.append(tp_t)

        # ---- heads ----
        for hi in range(hpc):
            qt2_sb = head_pool.tile([P, s], bf16, tag="q2")
            nc.sync.dma_start(qt2_sb[:], qt2_d[hi])
            qt1_sb = head_pool.tile([d + 1, s], bf16, tag="q1")
            nc.sync.dma_start(qt1_sb[:], qt1_d[hi])
            kt2_sb = head_pool.tile([P, sc], bf16, tag="k2")
            nc.sync.dma_start(kt2_sb[:], kt2_d[hi])
            kt1_sb = head_pool.tile([d + 1, sc], bf16, tag="k1")
            nc.sync.dma_start(kt1_sb[:], kt1_d[hi])
            v_sb = head_pool.tile([P, nkc * d], fp16, tag="v")
            nc.sync.dma_start(v_sb[:], v_d[hi])

            # vmfix row 0 = sum of V over non-compact (masked, unpadded) rows
            vmfix = head_pool.tile([P, d], fp16, tag="vmfix")
            nc.gpsimd.memset(vmfix[:], 0.0)
            if nrest > 0:
                vm_sb = head_pool.tile([P, nrest * d], fp16, tag="vmrest")
                nc.sync.dma_start(vm_sb[:], vm_d[hi])
                mv_ps = psum_o.tile([1, d], f32, tag="o_ps")
                for c in range(nrest):
                    nc.tensor.matmul(mv_ps[:], lhsT=onescol[:],
                                     rhs=vm_sb[:, c * d:(c + 1) * d],
                                     start=(c == 0), stop=(c == nrest - 1))
                nc.scalar.copy(vmfix[0:1, :], mv_ps[:])

            p_head = p_pool.tile([P, nqt, sc], f32, tag="p")
            o_head = o_pool.tile([P, nqt, d], f32, tag="o")

            def tile_front(qi):
                s_ps = psum_s.tile([P, sc], f32, tag="ps")
                for (off, w) in mh:
                    # qhi@khi + qlo@khi
                    nc.tensor.matmul(s_ps[:, off:off + w],
                                     lhsT=qt2_sb[:, qi * P:(qi + 1) * P],
                                     rhs=kt2_sb[:, off:off + w],
                                     start=True, stop=False)
                    # qhi@klo + C (aug row: ones x cK)
                    nc.tensor.matmul(s_ps[:, off:off + w],
                                     lhsT=qt1_sb[:, qi * P:(qi + 1) * P],
                                     rhs=kt1_sb[:, off:off + w],
                                     start=False, stop=True)
                return s_ps

            def tile_back(qi, s_ps):
                x = x_pool.tile([P, sc], f32, tag="x")
                nc.vector.tensor_tensor(x[:], s_ps[:], tp_tiles[qi][:], Alu.mult)
                e = e_pool.tile([P, sc], f32, tag="e")
                z = stat_pool.tile([P, 1], f32, tag="z")
                nc.scalar.activation(e[:], x[:], Act.Exp,
                                     bias=0.0, scale=1.0, accum_out=z[:])
                rz = stat_pool.tile([P, 1], f32, tag="rz")
                nc.vector.reciprocal(rz[:], z[:])
                # p (for the p_attn output) is off the critical path:
                nc.gpsimd.tensor_scalar_mul(p_head[:, qi, :], e[:], rz[:])

                # PV path transposes unnormalized e; normalize on [128,d].
                ha = (nkc + 1) // 2
                pt = pt_pool.tile([P, nkc, P], fp16, tag="pt")
                for h2 in range(2):
                    c0, c1 = (0, ha) if h2 == 0 else (ha, nkc)
                    if c0 >= c1:
                        continue
                    pt_ps = psum_t.tile([P, ha, P], f32, tag="pt_ps")
                    for c in range(c0, c1):
                        nc.tensor.transpose(pt_ps[:, c - c0, :],
                                            e[:, c * P:(c + 1) * P], ident[:])
                    # PSUM f32 -> SBUF fp16 cast; balance ACT/DVE
                    if h2 == 0:
                        nc.scalar.copy(pt[:, c0:c1, :], pt_ps[:, 0:c1 - c0, :])
                    else:
                        nc.vector.tensor_copy(pt[:, c0:c1, :],
                                              pt_ps[:, 0:c1 - c0, :])
                o_ps = psum_o.tile([P, d], f32, tag="o_ps")
                for c in range(nkc):
                    nc.tensor.matmul(o_ps[:], lhsT=pt[:, c, :],
                                     rhs=v_sb[:, c * d:(c + 1) * d],
                                     start=(c == 0), stop=False)
                # dead-row fixup: (1-mask_q) outer sum(V over non-compact rows)
                nc.tensor.matmul(o_ps[:], lhsT=deadrow[:, qi, :], rhs=vmfix[:],
                                 start=False, stop=True)
                # o = o_ps * rz * zsc  (zsc rescales dead rows by sc/s)
                nc.vector.tensor_scalar(o_head[:, qi, :], o_ps[:],
                                        rz[:], zsc[:, qi:qi + 1],
                                        Alu.mult, Alu.mult)

            # 1-deep software pipeline: QK(qi+1) issues before back(qi) so
            # the in-order PE stream never waits on ACT's exp.
            prev = None
            for qi in range(nqt):
                sp = tile_front(qi)
                if prev is not None:
                    tile_back(*prev)
                prev = (qi, sp)
            tile_back(*prev)
            # one batched store per head for p_attn and out
            nc.gpsimd.dma_start(
                p_out[hi].rearrange("(t p) k -> p t k", p=P), p_head[:])
            nc.gpsimd.dma_start(
                o_out[hi].rearrange("(t p) k -> p t k", p=P), o_head[:])


def build_nc(hpc=HPC, s=S, d=D, sc=SC):
    bass, tile, mybir = _concourse()
    f32 = mybir.dt.float32
    bf16 = mybir.dt.bfloat16
    fp16 = mybir.dt.float16
    nqt = s // P
    nkc = sc // P
    nrest = (s - sc) // P

    bacc = _bacc()
    nc = bacc.Bacc(trn_type="TRN2", target_bir_lowering=False, debug=False)
    names = {
        "qt2": ([hpc, P, s], bf16, "ExternalInput"),
        "qt1": ([hpc, d + 1, s], bf16, "ExternalInput"),
        "kt2": ([hpc, P, sc], bf16, "ExternalInput"),
        "kt1": ([hpc, d + 1, sc], bf16, "ExternalInput"),
        "v": ([hpc, P, nkc * d], fp16, "ExternalInput"),
        "vm": ([hpc, P, max(nrest, 1) * d], fp16, "ExternalInput"),
        "tq2": ([P, s], bf16, "ExternalInput"),
        "tqh1": ([d + 1, s], bf16, "ExternalInput"),
        "tk2": ([P, sc], bf16, "ExternalInput"),
        "tkl": ([d + 1, sc], bf16, "ExternalInput"),
        "mkrep": ([P, sc], f32, "ExternalInput"),
        "mqcol": ([P, nqt], f32, "ExternalInput"),
        "zsc": ([P, nqt], f32, "ExternalInput"),
        "ident": ([P, P], f32, "ExternalInput"),
        "deadrow": ([P, nqt, P], fp16, "ExternalInput"),
        "onescol": ([P, 1], fp16, "ExternalInput"),
        "p_out": ([hpc, s, sc], f32, "ExternalOutput"),
        "o_out": ([hpc, s, d], f32, "ExternalOutput"),
    }
    aps = {}
    for name, (shape, dt_, kind) in names.items():
        aps[name] = nc.dram_tensor(name, shape, dt_, kind=kind).ap()

    with tile.TileContext(nc) as tc:
        emit_kernel(tc, aps, hpc, s, d, sc)
    nc.finalize()
    return nc


def compact_index(maskf, s, sc):
    """Gathered column order: valid keys, then masked padding to sc.
    Returns (idx[sc], rest[s-sc]) -- rest are masked keys not on device."""
    valid = np.nonzero(maskf > 0.5)[0]
    masked = np.nonzero(maskf <= 0.5)[0]
    npad = sc - len(valid)
    assert npad >= 0, f"valid count {len(valid)} exceeds SC={sc}"
    idx = np.concatenate([valid, masked[:npad]])
    rest = masked[npad:]
    return idx, rest


def make_core_inputs(q_c, k_c, v_c, maskf, tq_b, tk_b, s=S, d=D, sc=SC):
    """Host-side data prep for one core.
    q_c,k_c,v_c: [hpc, s, d] f32; maskf: [s] f32; tq_b,tk_b: [s, d] f32."""
    hpc = q_c.shape[0]
    nqt = s // P
    nkc = sc // P
    nrest = (s - sc) // P
    idx, rest = compact_index(maskf, s, sc)
    mko = maskf[idx]                                   # compact mask (k side)

    def hilo(a):
        hi = a.astype(BF)
        lo = (a - hi.astype(np.float32)).astype(BF)
        return hi, lo

    qT = (q_c / 8.0).transpose(0, 2, 1)                # [hpc, d, s]
    kTc = k_c[:, idx, :].transpose(0, 2, 1)            # [hpc, d, sc]
    qhi, qlo = hilo(qT)
    khi, klo = hilo(kTc)
    qt2 = np.ascontiguousarray(np.concatenate([qhi, qlo], 1))
    kt2 = np.ascontiguousarray(np.concatenate([khi, khi], 1))
    ones_h = np.ones((hpc, 1, s), BF)
    ck = np.broadcast_to((NEG * (1.0 - mko)).astype(BF), (hpc, 1, sc))
    qt1 = np.ascontiguousarray(np.concatenate([qhi, ones_h], 1))
    kt1 = np.ascontiguousarray(np.concatenate([klo, ck], 1))

    vv = v_c[:, idx, :].astype(np.float16).reshape(hpc, nkc, P, d)
    vv = np.ascontiguousarray(vv.transpose(0, 2, 1, 3)).reshape(hpc, P, nkc * d)
    if nrest > 0:
        vm = v_c[:, rest, :].astype(np.float16).reshape(hpc, nrest, P, d)
        vm = np.ascontiguousarray(vm.transpose(0, 2, 1, 3)).reshape(
            hpc, P, nrest * d)
    else:
        vm = np.zeros((hpc, P, d), np.float16)

    tqs = tq_b / 8.0
    tqhi, tqlo = hilo(tqs.T)                           # [d, s]
    tkc = tk_b[idx, :].T                               # [d, sc]
    tkhi, tklo = hilo(tkc)
    ones_r = np.ones((1, s), BF)
    tq2 = np.ascontiguousarray(np.concatenate([tqhi, tqlo], 0))
    tqh1 = np.ascontiguousarray(np.concatenate([tqhi, ones_r], 0))
    tk2 = np.ascontiguousarray(np.concatenate([tkhi, tkhi], 0))
    tkl = np.ascontiguousarray(
        np.concatenate([tklo, -np.ones((1, sc), BF)], 0))

    mkrep = np.ascontiguousarray(np.broadcast_to(mko, (P, sc))).astype(np.float32)
    mqcol = np.ascontiguousarray(maskf.reshape(nqt, P).T).astype(np.float32)
    zsc = (mqcol + (1.0 - mqcol) * (float(sc) / s)).astype(np.float32)
    ident = np.eye(P, dtype=np.float32)
    deadrow = np.zeros((P, nqt, P), np.float16)
    deadrow[0] = (1.0 - maskf.reshape(nqt, P)).astype(np.float16)
    onescol = np.ones((P, 1), np.float16)
    return {
        "qt2": qt2, "kt2": kt2, "qt1": qt1, "kt1": kt1, "v": vv, "vm": vm,
        "tq2": tq2, "tqh1": tqh1, "tk2": tk2, "tkl": tkl,
        "mkrep": mkrep, "mqcol": mqcol, "zsc": zsc, "ident": ident,
        "deadrow": deadrow, "onescol": onescol,
    }


def assemble_p(p_compact, maskf, s=S, sc=SC):
    """Scatter compact p back to full [.., s, s]; fill dead rows/columns.
    p_compact: [..., s, sc]."""
    idx, _ = compact_index(maskf, s, sc)
    full = np.zeros(p_compact.shape[:-1] + (s,), np.float32)
    full[..., idx] = p_compact
    dead = np.nonzero(maskf <= 0.5)[0]
    full[..., dead, :] = np.float32(1.0 / s)
    return full


_NC_CACHE = {}


def _get_nc():
    if "nc" not in _NC_CACHE:
        _NC_CACHE["nc"] = build_nc()
    return _NC_CACHE["nc"]


def run_cores(in_maps, trace=False, trace_kwargs=None):
    from concourse.bass_utils import run_bass_kernel_spmd
    nc = _get_nc()
    kw = {}
    if trace:
        kw["trace"] = True
        if trace_kwargs:
            kw["trace_kwargs"] = trace_kwargs
    return run_bass_kernel_spmd(nc, in_maps, list(range(NCORES)), **kw)


def kernel(query, key, value, head_nums, mask, topic_query, topic_key,
           _trace=False):
    query = np.asarray(query, np.float32)
    key = np.asarray(key, np.float32)
    value = np.asarray(value, np.float32)
    mask = np.asarray(mask)
    topic_query = np.asarray(topic_query, np.float32)
    topic_key = np.asarray(topic_key, np.float32)

    in_maps = []
    for c in range(NCORES):
        b = c // 2
        h0 = (c % 2) * HPC
        in_maps.append(make_core_inputs(
            query[b, h0:h0 + HPC], key[b, h0:h0 + HPC], value[b, h0:h0 + HPC],
            mask[b].astype(np.float32), topic_query[b], topic_key[b]))

    res = run_cores(in_maps, trace=_trace)
    out = np.zeros((B, H, S, D), np.float32)
    p_attn = np.zeros((B, H, S, S), np.float32)
    for c in range(NCORES):
        b = c // 2
        h0 = (c % 2) * HPC
        out[b, h0:h0 + HPC] = res.results[c]["o_out"]
        p_attn[b, h0:h0 + HPC] = assemble_p(
            res.results[c]["p_out"], mask[b].astype(np.float32))
    if _trace:
        return (out, p_attn), res
    return out, p_attn
